# revision 51
# baseline (speedup 1.0000x reference)
"""Gemma GQA self-attention prefill on 8 TRN2 NeuronCores.

Sharding: core c owns KV head c and its two query heads {c, c+8}
(the reference maps q head H to kv head H % 8).  Each core computes
qT/kT/v projections for its slice directly in transposed layouts,
runs causal attention in the S^T formulation (keys on partitions),
then applies its own 512-row slice of W_o to all 2048 tokens and
writes a partial [2048, 3072] output.  The host sums the 8 partials
(the o_proj all-reduce is folded into the host-side unshard step).

No collectives; all matmuls in bf16 (fp32 accumulation in PSUM).

vs v1 (696us): killed the AllToAll + W_o restream (75us transition),
sliced prologue loads (PE starts ~15us instead of ~53us), 4-deep
qkv psum pipeline (hides the RoPE DVE chain), broadcast-colsum
matmul replaces the [1,512] reciprocal + BC chain, 3-deep score
banks hide exp latency, and diagonal attention tiles are narrowed
to the causal boundary.

vs v2 (430us graded / 361us measured):
- xt in DRAM is partition-major ([128, 4b, 8s, 3c, 512t]) so each b0
  piece is one 3KB-contiguous run per partition and batches b1-3 are
  single 24KB-per-partition DMAs; wqk m0 is split in two so the PE
  starts after ~0.8MB (first matmul ~12.3us instead of ~14.2us).
- bulk weights (wqk m2-5, wv, ident, wo) load through the ACT engine's
  HWDGE queue, gated on the b0 x-stream, so the group-0-critical sync
  queue runs at full HBM bandwidth.
- attention group order is h0: j0..j3 then h1: j3..j0: each group's
  ~5.4us DVE softmax-tail chain (SUM-bank copy -> 3.4us reciprocal ->
  norm muls) fits inside the next group's PE time, killing the s_norm
  and s_rc stalls between small groups.
- the first 3 S^T tiles are hoisted between b3's v-projection groups
  so ACT's exp pipeline is full when the AV loop starts.
- phase 3 runs j3-first (TCORD), rotates over 5 PSUM banks
  [P0,P1,P2,P6,P3] (P4/P5 stay with the last group's AV pair), and
  its PSUM->SBUF copies are split between ACT (first 10 + odd, since
  DVE still drains the last two softmax chains) and DVE, tracked by
  per-bank semaphores.
"""

import contextlib
import ctypes
import os
import sys
import types

import numpy as np


def _install_ntff_hook():
    """bass_utils under axon imports antenv.axon_hooks, which this image's
    antenv stub lacks.  Recreate the hook via ctypes on libaxon_pjrt."""
    if "antenv.axon_hooks" in sys.modules:
        return
    hook = None
    so_path = "/opt/axon/libaxon_pjrt.so"
    try:
        lib = ctypes.CDLL(so_path)
        if hasattr(lib, "axon_start_nrt_profile"):
            lib.axon_start_nrt_profile.argtypes = [
                ctypes.POINTER(ctypes.c_int64),
                ctypes.c_size_t,
            ]
            lib.axon_start_nrt_profile.restype = ctypes.c_int64
            lib.axon_stop_nrt_profile.argtypes = [ctypes.c_char_p]
            lib.axon_stop_nrt_profile.restype = ctypes.c_int64

            @contextlib.contextmanager
            def hook(output_dir, device_ids):
                import jax

                jax.devices()
                if device_ids:
                    ids = (ctypes.c_int64 * len(device_ids))(*device_ids)
                    rc = lib.axon_start_nrt_profile(ids, len(device_ids))
                else:
                    rc = lib.axon_start_nrt_profile(None, 0)
                if rc != 0:
                    raise RuntimeError(f"axon_start_nrt_profile rc={rc}")
                try:
                    yield
                finally:
                    n = lib.axon_stop_nrt_profile(str(output_dir).encode())
                    print(f"profile: {n} file(s) in {output_dir}", file=sys.stderr)

    except OSError:
        hook = None
    mod = types.ModuleType("antenv.axon_hooks")
    mod.get_axon_ntff_profile_hook = lambda: hook
    mod.set_axon_ntff_profile_hook = lambda h: None
    sys.modules["antenv.axon_hooks"] = mod


_install_ntff_hook()

import ml_dtypes  # noqa: E402
import concourse.bass as bass  # noqa: E402
import concourse.mybir as mybir  # noqa: E402
from concourse.bass_utils import run_bass_kernel_spmd  # noqa: E402

BF16 = mybir.dt.bfloat16
F32 = mybir.dt.float32

N_CORES = 8
T = 2048          # sequence length
HID = 3072        # hidden
KC = 24           # hidden chunks of 128
D = 256           # head dim

# attention tile lists: per local head h (0,1), t-tile j (4 of 512),
# u-tile i (16 of 128); causal keeps i <= 4j+3.  Tiles with i >= 4j sit on
# the causal diagonal: only q columns >= 128*(i-4j) of the 512-token window
# are live, and the first 128 live columns need the triangular mask.
# Group order h0: j0..j3 then h1: j3..j0 -- every group's ~5.4us DVE
# softmax-tail chain then fits inside the NEXT group's PE time, so the
# s_norm waits (AV-pair bank reuse two groups later) never stall the PE.
GROUPS = [(0, j) for j in range(4)] + [(1, j) for j in range(3, -1, -1)]
GIDX = {hj: g for g, hj in enumerate(GROUPS)}
TILES = [(h, j, i) for (h, j) in GROUPS for i in range(4 * j + 4)]
N_TILES = len(TILES)  # 80
GROUP_OF = {}
G_FIRST = {}
G_LAST = {}
for idx, (h, j, i) in enumerate(TILES):
    g = GIDX[(h, j)]
    GROUP_OF[idx] = g
    G_FIRST.setdefault(g, idx)
    G_LAST[g] = idx
OFFS = []  # live-column offset within the 512-token window (0 for full tiles)
for idx, (h, j, i) in enumerate(TILES):
    OFFS.append(128 * (i - 4 * j) if i >= 4 * j else 0)
# phase 3 processes token-tiles j3-first (those groups' norms finish first)
TCORD = [12, 13, 14, 15, 8, 9, 10, 11, 4, 5, 6, 7, 0, 1, 2, 3]



def build_program():
    nc = bass.Bass(trn_type="TRN2", num_devices=N_CORES)

    # xt is partition-major: xt[p, b, s, c, t] = x[512b+t, 384s+128c+p],
    # so each (b, s) piece is one 3KB-contiguous run per partition
    xt = nc.dram_tensor("xt", [128, 4, 8, 3, 512], BF16, kind="ExternalInput")
    # wqk is slab-major ([p, s, m, k3, c]): slab s holds kc 3s..3s+2 for all
    # six m-slices, so b0's piece-major qk loop consumes (slab s, piece s)
    # pairs as they stream in
    wqk = nc.dram_tensor("wqk", [128, 8, 6, 3, 128], BF16, kind="ExternalInput")
    wv = nc.dram_tensor("wv", [128, KC, 256], BF16, kind="ExternalInput")
    cosT = nc.dram_tensor("cosT", [128, T], F32, kind="ExternalInput")
    sinT = nc.dram_tensor("sinT", [128, T], F32, kind="ExternalInput")
    ident = nc.dram_tensor("ident", [128, 128], BF16, kind="ExternalInput")
    negtri = nc.dram_tensor("negtri", [128, 128], BF16, kind="ExternalInput")
    wo = nc.dram_tensor("wo", [512, HID], BF16, kind="ExternalInput")
    out = nc.dram_tensor("out", [T, HID], BF16, kind="ExternalOutput")

    ctx = contextlib.ExitStack()
    with ctx:
        # ---- SBUF ----
        xt_sb = ctx.enter_context(nc.sbuf_tensor("xt_sb", [128, 2, KC, 512], BF16))
        wqk_sb = ctx.enter_context(
            nc.sbuf_tensor("wqk_sb", [128, 8, 6, 3, 128], BF16)
        )
        wv_sb = ctx.enter_context(nc.sbuf_tensor("wv_sb", [128, KC, 256], BF16))
        cos_sb = ctx.enter_context(nc.sbuf_tensor("cos_sb", [128, T], F32))
        sin_sb = ctx.enter_context(nc.sbuf_tensor("sin_sb", [128, T], F32))
        id_sb = ctx.enter_context(nc.sbuf_tensor("id_sb", [128, 128], BF16))
        ntri_sb = ctx.enter_context(nc.sbuf_tensor("ntri_sb", [128, 128], BF16))
        ones_sb = ctx.enter_context(nc.sbuf_tensor("ones_sb", [128, 128], BF16))
        qk_sb = ctx.enter_context(nc.sbuf_tensor("qk_sb", [128, 6, T], BF16))
        v_sb = ctx.enter_context(nc.sbuf_tensor("v_sb", [128, 16, 256], BF16))
        pt_sb = ctx.enter_context(nc.sbuf_tensor("pt_sb", [128, 4, 512], BF16))
        rb_sb = ctx.enter_context(nc.sbuf_tensor("rb_sb", [128, 2, 512], F32))
        cs_sb = ctx.enter_context(nc.sbuf_tensor("cs_sb", [128, 2, 512], F32))
        tmpA = ctx.enter_context(nc.sbuf_tensor("tmpA", [128, 512], F32))
        tmpB = ctx.enter_context(nc.sbuf_tensor("tmpB", [128, 512], F32))
        ot_sb = ctx.enter_context(nc.sbuf_tensor("ot_sb", [128, 8, 2, 512], BF16))
        wo_sb = ctx.enter_context(nc.sbuf_tensor("wo_sb", [128, 4, HID], BF16))
        outst = ctx.enter_context(nc.sbuf_tensor("outst", [128, 4, 512], BF16))

        # ---- PSUM (8 full banks) ----
        P = [
            ctx.enter_context(nc.psum_tensor(f"ps{i}", [128, 512], F32))
            for i in range(8)
        ]
        # phase 1: qkT groups -> P[g%4]; v chunks -> P[4+vg%2][:, 0:256]
        # phase 2: ST -> P[idx%3]; AV pairs even g (P6,P3), odd g (P4,P5)
        #          (P6 is free from the start, so g0 never waits on the b3
        #          v-bank copies); broadcast colsum -> P7
        # phase 3: out tiles rotate [P0,P1,P2,P6,P3] -- P4/P5 stay with the
        #          last attention group's AV pair so phase 3 never waits on
        #          its norm chain
        SUMP = P[7]
        BANKS3 = [0, 1, 2, 6, 3]

        def avp(g, dcc):
            return P[[6, 3][dcc] if g % 2 == 0 else [4, 5][dcc]]

        # ---- semaphores ----
        sems = {}
        for name in (
            "s_wq", "s_wq0a",
            "s_x00", "s_x01", "s_x02", "s_x03",
            "s_x04", "s_x05", "s_x06", "s_x07",
            "s_wv", "s_xt1", "s_xt2", "s_xt3",
            "s_cs0", "s_cs1", "s_cs2", "s_cs3",
            "s_init", "s_wo", "s_misc", "s_pq", "s_pv", "s_pqd", "s_vcp",
            "s_dve", "s_stp", "s_exp", "s_ptc", "s_sum", "s_av",
            "s_rc", "s_rcp", "s_norm", "s_p3",
            "s_b30", "s_b31", "s_b32", "s_b33", "s_b34",
            "s_out0", "s_out1", "s_out2", "s_out3",
        ):
            sems[name] = ctx.enter_context(nc.semaphore(name))
        S = types.SimpleNamespace(**sems)
        s_x0 = [S.s_x00, S.s_x01, S.s_x02, S.s_x03,
                S.s_x04, S.s_x05, S.s_x06, S.s_x07]
        s_xt = [None, S.s_xt1, S.s_xt2, S.s_xt3]
        s_cs = [S.s_cs0, S.s_cs1, S.s_cs2, S.s_cs3]
        s_out = [S.s_out0, S.s_out1, S.s_out2, S.s_out3]
        # phase-3 copy-done sems, one per rotation bank (engine-agnostic)
        s_b3 = [S.s_b30, S.s_b31, S.s_b32, S.s_b33, S.s_b34]
        # phase-3 copy engine assignment: ACT takes the first 10 (DVE is
        # still draining the last two softmax-tail chains then) + odd q;
        # DVE takes even q >= 10
        P3_ON_ACT = [q for q in range(96) if q < 10 or q % 2 == 1]
        P3_ON_DVE = [q for q in range(96) if q >= 10 and q % 2 == 0]

        with nc.Block() as block:

            # ---------------- SYNC: weights + xt b1-3 + stores ----------------
            @block.sync
            def _(sync):
                def ld(sem, out_ap, in_ap):
                    sync.dma_start(out_ap, in_ap).then_inc(sem, 16)

                def xt_batch(b, sem):
                    # one DMA: per-partition 24KB fully contiguous
                    ld(sem, xt_sb[:, b % 2, :, :], xt[:, b, :, :, :])

                def cs_batch(b):
                    tsl = slice(512 * b, 512 * b + 512)
                    ld(s_cs[b], cos_sb[:, tsl], cosT[:, tsl])
                    ld(s_cs[b], sin_sb[:, tsl], sinT[:, tsl])

                # prologue: the b0-critical stream (wqk slabs + b0 x pieces,
                # consumed pairwise by the piece-major qk loop) runs alone on
                # the sync queue at full bandwidth; wv/cos0/ident/wo go on
                # the ACT HWDGE queue, gated until most of b0 has landed.
                for s in range(8):
                    if s == 0:
                        # split slab 0: the PE starts on (m0, piece0) after
                        # ~0.5MB instead of ~1MB
                        ld(S.s_wq0a, wqk_sb[:, 0, 0:1, :, :],
                           wqk[:, 0, 0:1, :, :])
                        sync.dma_start(
                            xt_sb[:, 0, 0:3, :], xt[:, 0, 0, :, :],
                        ).then_inc(s_x0[0], 16)
                        ld(S.s_wq, wqk_sb[:, 0, 1:6, :, :],
                           wqk[:, 0, 1:6, :, :])
                        continue
                    ld(S.s_wq, wqk_sb[:, s, :, :, :], wqk[:, s, :, :, :])
                    sync.dma_start(
                        xt_sb[:, 0, 3 * s:3 * s + 3, :],
                        xt[:, 0, s, :, :],
                    ).then_inc(s_x0[s], 16)
                # don't let the 3MB b1 load flood the queue before b0 is in
                sync.wait_ge(S.s_wq, 16 * 8)
                xt_batch(1, s_xt[1])
                cs_batch(1)
                sync.wait_ge(S.s_pq, 6)
                sync.wait_ge(S.s_pv, 4)
                xt_batch(2, s_xt[2])
                cs_batch(2)
                sync.wait_ge(S.s_pq, 12)
                sync.wait_ge(S.s_pv, 8)
                xt_batch(3, s_xt[3])
                cs_batch(3)

                # phase 3 output stores
                for q in range(96):
                    sync.wait_ge(s_b3[q % 5], q // 5 + 1)
                    n, tc = q // 16, TCORD[q % 16]
                    sync.dma_start(
                        out[128 * tc:128 * tc + 128, 512 * n:512 * n + 512],
                        outst[:, q % 4, :],
                    ).then_inc(s_out[q % 4], 16)

            # ---------------- GPSIMD ----------------
            @block.gpsimd
            def _(gp):
                gp.memset(ones_sb[:, :], 1.0).then_inc(S.s_misc, 1)

            # ---------------- TENSOR (PE) ----------------
            @block.tensor
            def _(pe):
                # phase 2 tile helpers (defined first: the last 3 v groups
                # of phase 1 interleave the first S^T tiles)
                def emit_st(idx):
                    h, j, i = TILES[idx]
                    o = OFFS[idx]
                    diag = i >= 4 * j
                    if idx == 0:
                        # P0/P1 bank WAR vs RoPE pair 10 (reads qk banks
                        # P0/P1 of g20/g21); also covers all j0 q/k data
                        pe.wait_ge(S.s_pqd, 22)
                        pe.wait_ge(S.s_init, 16 * 2)  # ident + negtri
                    elif idx == 2:
                        # P2 bank WAR vs RoPE pair 11 (the last pair)
                        pe.wait_ge(S.s_pqd, 24)
                    if idx >= 3:
                        # ST bank WAR vs exp(idx-3); the wait value idx-2
                        # also covers cons(idx-3)'s exp data dependency
                        pe.wait_ge(S.s_exp, idx - 2)
                    for dc in range(2):
                        ins = pe.matmul(
                            P[idx % 3][:, o:512],
                            lhsT=qk_sb[:, 4 + dc, 128 * i:128 * i + 128],
                            rhs=qk_sb[:, 2 * h + dc, 512 * j + o:512 * j + 512],
                            start=(dc == 0),
                            stop=(dc == 1 and not diag),
                        )
                    if diag:
                        # add -30000 to the causal-masked triangle so exp
                        # underflows to zero -- replaces the DVE mask multiply
                        ins = pe.matmul(
                            P[idx % 3][:, o:o + 128],
                            lhsT=id_sb[:, :],
                            rhs=ntri_sb[:, :],
                            start=False,
                            stop=True,
                        )
                    ins.then_inc(S.s_stp, 1)

                # phase 1, b0: piece-major (s outer, all six m inner, banks
                # P0-P5) -- the PE starts on (slab 0, piece 0) after ~1MB
                # and the whole 7.9MB b0 load overlaps b0's 30.7us of qk
                # compute instead of serializing ahead of it
                for s in range(8):
                    pe.wait_ge(s_x0[s], 16)
                    if s > 0:
                        pe.wait_ge(S.s_wq, 16 * (s + 1))
                    for m in range(6):
                        if s == 0:
                            if m == 0:
                                pe.wait_ge(S.s_wq0a, 16)
                            elif m == 1:
                                pe.wait_ge(S.s_wq, 16)
                        for k3 in range(3):
                            ins = pe.matmul(
                                P[m][:, :],
                                lhsT=wqk_sb[:, s, m, k3, :],
                                rhs=xt_sb[:, 0, 3 * s + k3, :],
                                start=(s == 0 and k3 == 0),
                                stop=(s == 7 and k3 == 2),
                            )
                        if s == 7:
                            ins.then_inc(S.s_pq, 1)
                # v(b0) on P6/P7 (free in phase 1; P0-P5 hold b0's qk until
                # the RoPE pairs drain them)
                for ts in range(4):
                    if ts == 0:
                        pe.wait_ge(S.s_wv, 16)
                    else:
                        pe.wait_ge(S.s_vcp, max(ts - 1, 0))
                    for kc in range(KC):
                        ins = pe.matmul(
                            P[6 + ts % 2][:, 0:256],
                            lhsT=xt_sb[:, 0, kc, 128 * ts:128 * ts + 128],
                            rhs=wv_sb[:, kc, :],
                            start=(kc == 0),
                            stop=(kc == KC - 1),
                        )
                    ins.then_inc(S.s_pv, 1)

                # phase 1, b1-3: group-major (weights fully resident)
                for b in range(1, 4):
                    for m in range(6):
                        g = 6 * b + m
                        # bank g%4 was read by the RoPE pair containing
                        # group g-4; that pair completes at s_pqd = g-2
                        # for even g (pair g-4,g-3) and g-3 for odd g
                        # (pair g-5,g-4)
                        pe.wait_ge(S.s_pqd, g - 2 if g % 2 == 0 else g - 3)
                        if m == 0:
                            pe.wait_ge(s_xt[b], 16)
                        for kc in range(KC):
                            ins = pe.matmul(
                                P[g % 4][:, :],
                                lhsT=wqk_sb[:, kc // 3, m, kc % 3, :],
                                rhs=xt_sb[:, b % 2, kc, :],
                                start=(kc == 0),
                                stop=(kc == KC - 1),
                            )
                        ins.then_inc(S.s_pq, 1)
                    for ts in range(4):
                        vg = 4 * b + ts
                        if b == 1 and ts == 0:
                            # P4/P5 were b0's k-groups: RoPE pair 2 must
                            # have drained them before v(b1) overwrites
                            pe.wait_ge(S.s_pqd, 6)
                        pe.wait_ge(S.s_vcp, vg - 1)
                        for kc in range(KC):
                            ins = pe.matmul(
                                P[4 + vg % 2][:, 0:256],
                                lhsT=xt_sb[:, b % 2, kc, 128 * ts:128 * ts + 128],
                                rhs=wv_sb[:, kc, :],
                                start=(kc == 0),
                                stop=(kc == KC - 1),
                            )
                        ins.then_inc(S.s_pv, 1)
                        # hoist the first 3 attention S^T tiles between b3's
                        # v groups: their exps fill the ACT pipeline while
                        # the PE finishes phase 1, so the cons loop starts
                        # with zero exp-latency bubble
                        if b == 3:
                            if ts == 0:
                                emit_st(0)
                                emit_st(1)
                            elif ts == 1:
                                emit_st(2)

                def emit_sum(idx):
                    # SUM of tile idx is deferred one tile so the previous
                    # group's SUM-bank evacuation never blocks the PE; it is
                    # also the last reader of pt slot idx (-> s_ptc)
                    g = GROUP_OF[idx]
                    o = OFFS[idx]
                    first = idx == G_FIRST[g]
                    last = idx == G_LAST[g]
                    if idx == 0:
                        pe.wait_ge(S.s_misc, 1)  # ones_sb memset
                    if first:
                        pe.wait_ge(S.s_rc, g)  # SUM bank free (g=0 trivial)
                    sm = pe.matmul(
                        SUMP[:, o:512], lhsT=ones_sb[:, :],
                        rhs=pt_sb[:, idx % 4, o:512],
                        start=first, stop=last,
                    )
                    # one sem update per instruction: group-last SUM signals
                    # s_sum (softmax tail); others signal s_ptc (pt slot)
                    if last:
                        sm.then_inc(S.s_sum, 1)
                    else:
                        sm.then_inc(S.s_ptc, 1)

                vcp_seen = [0]

                def emit_cons(idx):
                    h, j, i = TILES[idx]
                    o = OFFS[idx]
                    g = GROUP_OF[idx]
                    first = idx == G_FIRST[g]
                    last = idx == G_LAST[g]
                    if idx + 3 >= N_TILES:
                        # no emit_st carries this tile's exp wait
                        pe.wait_ge(S.s_exp, idx + 1)
                    if idx == 0:
                        need = 4  # v tiles 0-3 (g0 is all of j0)
                    elif idx == 4:
                        need = 16  # P4/P5 bank WAR vs the b3 v-bank copies
                    else:
                        need = i + 1  # v_sb tile i data
                    if need > vcp_seen[0]:
                        pe.wait_ge(S.s_vcp, need)
                        vcp_seen[0] = need
                    if first and g >= 2:
                        pe.wait_ge(S.s_norm, 2 * g - 2)  # AV pair free
                    pt = pt_sb[:, idx % 4, o:512]
                    av = [
                        pe.matmul(
                            avp(g, dc)[:, o:512],
                            lhsT=v_sb[:, i, 128 * dc:128 * dc + 128],
                            rhs=pt,
                            start=first,
                            stop=last,
                        )
                        for dc in range(2)
                    ]
                    if last:
                        av[1].then_inc(S.s_av, 1)

                # st(0..2) were interleaved into phase 1's b3 v groups
                for idx in range(N_TILES):
                    if idx + 3 < N_TILES:
                        emit_st(idx + 3)
                    emit_cons(idx)
                    if idx >= 1:
                        emit_sum(idx - 1)
                    if idx == N_TILES - 1:
                        emit_sum(idx)

                # phase 3: local o_proj (K=512) over all 2048 tokens,
                # j3-first token order, 5-bank rotation [P0,P1,P2,P6,P3]
                for q in range(96):
                    n, tc = q // 16, TCORD[q % 16]
                    # data: tile tc needs groups GIDX[(0,j)], GIDX[(1,j)]
                    # for j = tc//4: j3 -> norm 10, j2 -> 12, j1 -> 14,
                    # j0 -> 16.  banks: P6/P3 are g6's AV pair (free at
                    # s_norm 13/14); P4/P5 (g7's) are never used here.
                    if q == 0:
                        pe.wait_ge(S.s_norm, 10)
                        pe.wait_ge(S.s_wo, 16)
                    elif q == 3:
                        pe.wait_ge(S.s_norm, 13)
                    elif q == 4:
                        pe.wait_ge(S.s_norm, 14)
                    elif q == 12:
                        pe.wait_ge(S.s_norm, 16)
                    if q >= 5:
                        # bank reuse: tile q-5's copy must have evacuated it
                        pe.wait_ge(s_b3[q % 5], q // 5)
                    bank = P[BANKS3[q % 5]]
                    for c2 in range(4):
                        h, dcc = divmod(c2, 2)
                        ins = pe.matmul(
                            bank[:, :],
                            lhsT=ot_sb[:, GIDX[(h, tc // 4)], dcc,
                                       128 * (tc % 4):128 * (tc % 4) + 128],
                            rhs=wo_sb[:, c2, 512 * n:512 * n + 512],
                            start=(c2 == 0),
                            stop=(c2 == 3),
                        )
                    ins.then_inc(S.s_p3, 1)

            # ---------------- VECTOR (DVE) ----------------
            @block.vector
            def _(ve):
                dvec = [0]  # same-engine serialization counter for temps

                def step(fn, *args, inc=None, inc_by=1):
                    if dvec[0]:
                        ve.wait_ge(S.s_dve, dvec[0])
                    ins = fn(*args)
                    if inc is None:
                        ins.then_inc(S.s_dve, 1)
                        dvec[0] += 1
                    else:
                        ins.then_inc(inc, inc_by)

                # phase 1: RoPE + v copies
                def rope_pair(b, p):
                    tsl = slice(512 * b, 512 * b + 512)
                    m = 2 * p
                    g0, g1 = 6 * b + m, 6 * b + m + 1
                    ve.wait_ge(S.s_pq, g1 + 1)
                    if p == 0:
                        ve.wait_ge(s_cs[b], 16 * 2)
                    if not (b == 0 and p == 0):
                        # tmpA/tmpB WAR vs the previous pair's final add
                        # (which increments s_pqd): DVE ops can pipeline,
                        # so an explicit wait is required
                        ve.wait_ge(S.s_pqd, 2 * (3 * b + p))
                    if b == 0:
                        # piece-major b0: group m lives in bank P[m]
                        q1, q2 = P[m][:, :], P[m + 1][:, :]
                    else:
                        q1, q2 = P[g0 % 4][:, :], P[g1 % 4][:, :]
                    step(ve.tensor_mul, tmpA[:, :], q1, cos_sb[:, tsl])
                    step(ve.tensor_mul, tmpB[:, :], q2, sin_sb[:, tsl])
                    step(ve.tensor_sub, qk_sb[:, m, tsl], tmpA[:, :],
                         tmpB[:, :])
                    step(ve.tensor_mul, tmpA[:, :], q2, cos_sb[:, tsl])
                    step(ve.tensor_mul, tmpB[:, :], q1, sin_sb[:, tsl])
                    step(ve.tensor_add, qk_sb[:, m + 1, tsl], tmpA[:, :],
                         tmpB[:, :], inc=S.s_pqd, inc_by=2)

                def v_copy(vg):
                    bank = P[6 + vg % 2] if vg < 4 else P[4 + vg % 2]
                    ve.wait_ge(S.s_pv, vg + 1)
                    ve.tensor_copy(v_sb[:, vg, :], bank[:, 0:256]).then_inc(
                        S.s_vcp, 1
                    )

                # b0: its v phase follows qk immediately (all RoPE becomes
                # runnable at once), so interleave copies with the pairs or
                # the P6/P7 double-buffer stalls the PE.  b1/b2: copies sit
                # between pair1 and pair2 for the same reason.  b3: pairs
                # first -- the hoisted st(2) needs pair 11 as early as
                # possible, and the interleaved S^T tiles give the DVE slack.
                rope_pair(0, 0)
                v_copy(0)
                v_copy(1)
                rope_pair(0, 1)
                v_copy(2)
                v_copy(3)
                rope_pair(0, 2)
                for b in (1, 2):
                    rope_pair(b, 0)
                    rope_pair(b, 1)
                    v_copy(4 * b + 0)
                    v_copy(4 * b + 1)
                    rope_pair(b, 2)
                    v_copy(4 * b + 2)
                    v_copy(4 * b + 3)
                rope_pair(3, 0)
                rope_pair(3, 1)
                rope_pair(3, 2)
                for ts in range(4):
                    v_copy(12 + ts)

                # phase 2: per-group softmax tail.  The copy evacuates the
                # SUM bank fast (s_rc) so the next group's SUMs never wait;
                # the slow reciprocal (3.4us) runs off the PE critical path.
                # The descending-j h1 group order guarantees each group's
                # chain fits inside the next group's PE time, so s_norm is
                # always ready when the AV pair is reused two groups later.
                for g in range(8):
                    ve.wait_ge(S.s_sum, g + 1)
                    ve.tensor_copy(cs_sb[:, g % 2, :], SUMP[:, :]).then_inc(
                        S.s_rc, 1
                    )
                    ve.wait_ge(S.s_rc, g + 1)  # cs RAW vs the copy above
                    ve.reciprocal(rb_sb[:, g % 2, :],
                                  cs_sb[:, g % 2, :]).then_inc(S.s_rcp, 1)
                    ve.wait_ge(S.s_av, g + 1)
                    ve.wait_ge(S.s_rcp, g + 1)  # rb RAW vs the reciprocal
                    ve.tensor_mul(ot_sb[:, g, 0, :], avp(g, 0)[:, :],
                                  rb_sb[:, g % 2, :]).then_inc(S.s_norm, 1)
                    ve.tensor_mul(ot_sb[:, g, 1, :], avp(g, 1)[:, :],
                                  rb_sb[:, g % 2, :]).then_inc(S.s_norm, 1)

                # phase 3: DVE's share of the output copies
                for q in P3_ON_DVE:
                    ve.wait_ge(S.s_p3, q + 1)
                    if q >= 4:
                        ve.wait_ge(s_out[q % 4], 16 * (q // 4))
                    ve.tensor_copy(outst[:, q % 4, :],
                                   P[BANKS3[q % 5]][:, :]).then_inc(
                        s_b3[q % 5], 1
                    )

            # ---------------- SCALAR (ACT): bulk DMAs + exp + copies -------
            @block.scalar
            def _(sc):
                # prologue: wv (needed at v(b0), right after the qk slabs)
                # and cos/sin b0 (needed at RoPE pair 0) on the ACT HWDGE
                # queue, gated so they only take bandwidth from the last
                # two slab/piece pairs
                sc.wait_ge(s_x0[5], 16)
                sc.dma_start(cos_sb[:, 0:512], cosT[:, 0:512]).then_inc(
                    s_cs[0], 16)
                sc.dma_start(sin_sb[:, 0:512], sinT[:, 0:512]).then_inc(
                    s_cs[0], 16)
                sc.dma_start(wv_sb[:, :, :], wv[:, :, :]).then_inc(S.s_wv, 16)
                sc.dma_start(id_sb[:, :], ident[:, :]).then_inc(S.s_init, 16)
                sc.dma_start(ntri_sb[:, :], negtri[:, :]).then_inc(S.s_init, 16)
                sc.dma_start(
                    wo_sb[:, :, :],
                    wo[:, :].rearrange("(c p) n -> p c n", p=128),
                ).then_inc(S.s_wo, 16)

                # cumulative count of non-group-last tiles (s_ptc increments)
                ptc_at = []
                c = 0
                for t in range(N_TILES):
                    if t != G_LAST[GROUP_OF[t]]:
                        c += 1
                    ptc_at.append(c)
                for idx in range(N_TILES):
                    o = OFFS[idx]
                    sc.wait_ge(S.s_stp, idx + 1)
                    if idx >= 4:
                        lo = idx - 4  # pt slot owner
                        if lo == G_LAST[GROUP_OF[lo]]:
                            sc.wait_ge(S.s_sum, GROUP_OF[lo] + 1)
                        else:
                            sc.wait_ge(S.s_ptc, ptc_at[lo])
                    sc.activation(
                        pt_sb[:, idx % 4, o:512],
                        P[idx % 3][:, o:512],
                        mybir.ActivationFunctionType.Exp,
                        scale=0.0625,
                    ).then_inc(S.s_exp, 1)

                # phase 3: ACT's share of the output copies
                for q in P3_ON_ACT:
                    sc.wait_ge(S.s_p3, q + 1)
                    if q >= 4:
                        sc.wait_ge(s_out[q % 4], 16 * (q // 4))
                    sc.copy(outst[:, q % 4, :],
                            P[BANKS3[q % 5]][:, :]).then_inc(
                        s_b3[q % 5], 1
                    )

    return nc


# ---------------- host side ----------------

NUM_HEADS = 16
NUM_KV_HEADS = 8
HEAD_DIM = 256
ROPE_THETA = 10000.0


def _prep(x, W_qkv, W_o):
    bf = ml_dtypes.bfloat16
    # xt[p, b, s, c, t] = x[512b+t, 384s+128c+p]: partition-major so each
    # (b, s) piece is a single 3KB-contiguous run per partition
    xt = np.ascontiguousarray(
        x.reshape(4, 512, 8, 3, 128).transpose(4, 0, 2, 3, 1)
    ).astype(bf)

    pos = np.arange(T, dtype=np.float64)
    inv_freq = 1.0 / ROPE_THETA ** (
        np.arange(0, HEAD_DIM, 2, dtype=np.float64) / HEAD_DIM
    )
    freqs = pos[:, None] * inv_freq[None, :]  # [T, 128]
    cosT = np.ascontiguousarray(np.cos(freqs).T).astype(np.float32)
    sinT = np.ascontiguousarray(np.sin(freqs).T).astype(np.float32)

    p = np.arange(128)[:, None]
    f = np.arange(128)[None, :]
    ident = np.eye(128, dtype=np.float32).astype(bf)
    negtri = np.where(f < p, -30000.0, 0.0).astype(np.float32).astype(bf)

    in_maps = []
    for c in range(N_CORES):
        q_cols = np.r_[
            HEAD_DIM * c:HEAD_DIM * (c + 1),
            HEAD_DIM * (c + 8):HEAD_DIM * (c + 9),
        ]
        k_cols = np.arange(
            HEAD_DIM * NUM_HEADS + HEAD_DIM * c,
            HEAD_DIM * NUM_HEADS + HEAD_DIM * (c + 1),
        )
        v_cols = np.arange(
            HEAD_DIM * (NUM_HEADS + NUM_KV_HEADS) + HEAD_DIM * c,
            HEAD_DIM * (NUM_HEADS + NUM_KV_HEADS) + HEAD_DIM * (c + 1),
        )
        # partition-major shuffles for long contiguous DMA lines:
        # wqk[p, s, m, k3, col] = W[128*(3s+k3)+p, 128m+col] (slab-major);
        # wv[p, c, col] = Wv[128c+p, col]
        wqk = np.ascontiguousarray(
            W_qkv[:, np.r_[q_cols, k_cols]]
            .reshape(KC, 128, 6, 128)
            .transpose(1, 2, 0, 3)
            .reshape(128, 6, 8, 3, 128)
            .transpose(0, 2, 1, 3, 4)
        ).astype(bf)
        wvc = np.ascontiguousarray(
            W_qkv[:, v_cols].reshape(KC, 128, 256).transpose(1, 0, 2)
        ).astype(bf)
        woc = np.ascontiguousarray(
            W_o[np.r_[HEAD_DIM * c:HEAD_DIM * (c + 1),
                      HEAD_DIM * (c + 8):HEAD_DIM * (c + 9)], :]
        ).astype(bf)
        in_maps.append(
            {
                "wqk": wqk,
                "wv": wvc,
                "wo": woc,
                "xt": xt,
                "cosT": cosT,
                "sinT": sinT,
                "ident": ident,
                "negtri": negtri,
            }
        )
    return in_maps


_CACHE = {}


def kernel(x, W_qkv, W_o):
    trace = bool(int(os.environ.get("KERNEL_TRACE", "0")))
    in_maps = _prep(
        np.asarray(x, np.float32),
        np.asarray(W_qkv, np.float32),
        np.asarray(W_o, np.float32),
    )
    if "nc" not in _CACHE:
        _CACHE["nc"] = build_program()
    nc = _CACHE["nc"]
    res = run_bass_kernel_spmd(
        nc, in_maps, list(range(N_CORES)), trace=trace,
        trace_cores=[0] if trace else None,
    )
    if trace:
        print(f"HW exec time: {res.exec_time_ns} ns")
        _CACHE["last_result"] = res
    acc = np.zeros((T, HID), dtype=np.float32)
    for c in range(N_CORES):
        acc += np.asarray(res.results[c]["out"], dtype=np.float32)
    return acc


if __name__ == "__main__":
    rng = np.random.default_rng(0)
    x = rng.standard_normal((T, HID), dtype=np.float32)
    Wq = (rng.standard_normal((HID, 8192), dtype=np.float32) * HID ** -0.5)
    Wo = (rng.standard_normal((4096, HID), dtype=np.float32) * 4096 ** -0.5)
    y = kernel(x, Wq, Wo)
    print("ran:", y.shape, y.dtype)



# revision 56
# speedup vs baseline: 1.1900x; 1.1900x over previous
"""Gemma GQA self-attention prefill on 8 TRN2 NeuronCores.

Sharding: core c owns KV head c and its two query heads {c, c+8}
(the reference maps q head H to kv head H % 8).  Each core computes
qT/kT/v projections for its slice directly in transposed layouts,
runs causal attention in the S^T formulation (keys on partitions),
then applies its own 512-row slice of W_o to all 2048 tokens and
writes a partial [2048, 3072] output.  The host sums the 8 partials
(the o_proj all-reduce is folded into the host-side unshard step).

No collectives; all matmuls in bf16 (fp32 accumulation in PSUM).

vs v1 (696us): killed the AllToAll + W_o restream (75us transition),
sliced prologue loads (PE starts ~15us instead of ~53us), 4-deep
qkv psum pipeline (hides the RoPE DVE chain), broadcast-colsum
matmul replaces the [1,512] reciprocal + BC chain, 3-deep score
banks hide exp latency, and diagonal attention tiles are narrowed
to the causal boundary.

vs v2 (430us graded / 361us measured):
- xt in DRAM is partition-major ([128, 4b, 8s, 3c, 512t]) so each b0
  piece is one 3KB-contiguous run per partition and batches b1-3 are
  single 24KB-per-partition DMAs; wqk m0 is split in two so the PE
  starts after ~0.8MB (first matmul ~12.3us instead of ~14.2us).
- bulk weights (wqk m2-5, wv, ident, wo) load through the ACT engine's
  HWDGE queue, gated on the b0 x-stream, so the group-0-critical sync
  queue runs at full HBM bandwidth.
- attention group order is h0: j0..j3 then h1: j3..j0: each group's
  ~5.4us DVE softmax-tail chain (SUM-bank copy -> 3.4us reciprocal ->
  norm muls) fits inside the next group's PE time, killing the s_norm
  and s_rc stalls between small groups.
- the first 3 S^T tiles are hoisted between b3's v-projection groups
  so ACT's exp pipeline is full when the AV loop starts.
- phase 3 runs j3-first (TCORD), rotates over 5 PSUM banks
  [P0,P1,P2,P6,P3] (P4/P5 stay with the last group's AV pair), and
  its PSUM->SBUF copies are split between ACT (first 10 + odd, since
  DVE still drains the last two softmax chains) and DVE, tracked by
  per-bank semaphores.
"""

import contextlib
import ctypes
import os
import sys
import types

import numpy as np


def _install_ntff_hook():
    """bass_utils under axon imports antenv.axon_hooks, which this image's
    antenv stub lacks.  Recreate the hook via ctypes on libaxon_pjrt."""
    if "antenv.axon_hooks" in sys.modules:
        return
    hook = None
    so_path = "/opt/axon/libaxon_pjrt.so"
    try:
        lib = ctypes.CDLL(so_path)
        if hasattr(lib, "axon_start_nrt_profile"):
            lib.axon_start_nrt_profile.argtypes = [
                ctypes.POINTER(ctypes.c_int64),
                ctypes.c_size_t,
            ]
            lib.axon_start_nrt_profile.restype = ctypes.c_int64
            lib.axon_stop_nrt_profile.argtypes = [ctypes.c_char_p]
            lib.axon_stop_nrt_profile.restype = ctypes.c_int64

            @contextlib.contextmanager
            def hook(output_dir, device_ids):
                import jax

                jax.devices()
                if device_ids:
                    ids = (ctypes.c_int64 * len(device_ids))(*device_ids)
                    rc = lib.axon_start_nrt_profile(ids, len(device_ids))
                else:
                    rc = lib.axon_start_nrt_profile(None, 0)
                if rc != 0:
                    raise RuntimeError(f"axon_start_nrt_profile rc={rc}")
                try:
                    yield
                finally:
                    n = lib.axon_stop_nrt_profile(str(output_dir).encode())
                    print(f"profile: {n} file(s) in {output_dir}", file=sys.stderr)

    except OSError:
        hook = None
    mod = types.ModuleType("antenv.axon_hooks")
    mod.get_axon_ntff_profile_hook = lambda: hook
    mod.set_axon_ntff_profile_hook = lambda h: None
    sys.modules["antenv.axon_hooks"] = mod


_install_ntff_hook()

import ml_dtypes  # noqa: E402
import concourse.bass as bass  # noqa: E402
import concourse.mybir as mybir  # noqa: E402
from concourse.bass_utils import run_bass_kernel_spmd  # noqa: E402

BF16 = mybir.dt.bfloat16
F32 = mybir.dt.float32

N_CORES = 8
T = 2048          # sequence length
HID = 3072        # hidden
KC = 24           # hidden chunks of 128
D = 256           # head dim

# attention tile lists: per local head h (0,1), t-tile j (4 of 512),
# u-tile i (16 of 128); causal keeps i <= 4j+3.  Tiles with i >= 4j sit on
# the causal diagonal: only q columns >= 128*(i-4j) of the 512-token window
# are live, and the first 128 live columns need the triangular mask.
# Group order h0: j0..j3 then h1: j3..j0 -- every group's ~5.4us DVE
# softmax-tail chain then fits inside the NEXT group's PE time, so the
# s_norm waits (AV-pair bank reuse two groups later) never stall the PE.
GROUPS = [(0, j) for j in range(4)] + [(1, j) for j in range(3, -1, -1)]
GIDX = {hj: g for g, hj in enumerate(GROUPS)}
TILES = [(h, j, i) for (h, j) in GROUPS for i in range(4 * j + 4)]
N_TILES = len(TILES)  # 80
GROUP_OF = {}
G_FIRST = {}
G_LAST = {}
for idx, (h, j, i) in enumerate(TILES):
    g = GIDX[(h, j)]
    GROUP_OF[idx] = g
    G_FIRST.setdefault(g, idx)
    G_LAST[g] = idx
OFFS = []  # live-column offset within the 512-token window (0 for full tiles)
for idx, (h, j, i) in enumerate(TILES):
    OFFS.append(128 * (i - 4 * j) if i >= 4 * j else 0)
# phase 3 processes token-tiles j3-first (those groups' norms finish first)
TCORD = [12, 13, 14, 15, 8, 9, 10, 11, 4, 5, 6, 7, 0, 1, 2, 3]



def build_program():
    nc = bass.Bass(trn_type="TRN2", num_devices=N_CORES)

    # xt is partition-major: xt[p, b, s, c, t] = x[512b+t, 384s+128c+p],
    # so each (b, s) piece is one 3KB-contiguous run per partition
    xt = nc.dram_tensor("xt", [128, 4, 8, 3, 512], BF16, kind="ExternalInput")
    # wqk is slab-major ([p, s, m, k3, c]): slab s holds kc 3s..3s+2 for all
    # six m-slices, so b0's piece-major qk loop consumes (slab s, piece s)
    # pairs as they stream in
    wqk = nc.dram_tensor("wqk", [128, 8, 6, 3, 128], BF16, kind="ExternalInput")
    wv = nc.dram_tensor("wv", [128, KC, 256], BF16, kind="ExternalInput")
    cosT = nc.dram_tensor("cosT", [128, T], F32, kind="ExternalInput")
    sinT = nc.dram_tensor("sinT", [128, T], F32, kind="ExternalInput")
    ident = nc.dram_tensor("ident", [128, 128], BF16, kind="ExternalInput")
    negtri = nc.dram_tensor("negtri", [128, 128], BF16, kind="ExternalInput")
    wo = nc.dram_tensor("wo", [512, HID], BF16, kind="ExternalInput")
    out = nc.dram_tensor("out", [T, HID], BF16, kind="ExternalOutput")

    ctx = contextlib.ExitStack()
    with ctx:
        # ---- SBUF ----
        xt_sb = ctx.enter_context(nc.sbuf_tensor("xt_sb", [128, 2, KC, 512], BF16))
        wqk_sb = ctx.enter_context(
            nc.sbuf_tensor("wqk_sb", [128, 8, 6, 3, 128], BF16)
        )
        wv_sb = ctx.enter_context(nc.sbuf_tensor("wv_sb", [128, KC, 256], BF16))
        cos_sb = ctx.enter_context(nc.sbuf_tensor("cos_sb", [128, T], F32))
        sin_sb = ctx.enter_context(nc.sbuf_tensor("sin_sb", [128, T], F32))
        id_sb = ctx.enter_context(nc.sbuf_tensor("id_sb", [128, 128], BF16))
        ntri_sb = ctx.enter_context(nc.sbuf_tensor("ntri_sb", [128, 128], BF16))
        ones_sb = ctx.enter_context(nc.sbuf_tensor("ones_sb", [128, 128], BF16))
        qk_sb = ctx.enter_context(nc.sbuf_tensor("qk_sb", [128, 6, T], BF16))
        v_sb = ctx.enter_context(nc.sbuf_tensor("v_sb", [128, 16, 256], BF16))
        pt_sb = ctx.enter_context(nc.sbuf_tensor("pt_sb", [128, 4, 512], BF16))
        rb_sb = ctx.enter_context(nc.sbuf_tensor("rb_sb", [128, 2, 512], F32))
        cs_sb = ctx.enter_context(nc.sbuf_tensor("cs_sb", [128, 2, 512], F32))
        tmpA = ctx.enter_context(nc.sbuf_tensor("tmpA", [128, 512], F32))
        tmpB = ctx.enter_context(nc.sbuf_tensor("tmpB", [128, 512], F32))
        ot_sb = ctx.enter_context(nc.sbuf_tensor("ot_sb", [128, 8, 2, 512], BF16))
        wo_sb = ctx.enter_context(nc.sbuf_tensor("wo_sb", [128, 4, HID], BF16))
        outst = ctx.enter_context(nc.sbuf_tensor("outst", [128, 4, 512], BF16))

        # ---- PSUM (8 full banks) ----
        P = [
            ctx.enter_context(nc.psum_tensor(f"ps{i}", [128, 512], F32))
            for i in range(8)
        ]
        # phase 1: qkT groups -> P[g%4]; v chunks -> P[4+vg%2][:, 0:256]
        # phase 2: ST -> P[idx%3]; AV pairs even g (P6,P3), odd g (P4,P5)
        #          (P6 is free from the start, so g0 never waits on the b3
        #          v-bank copies); broadcast colsum -> P7
        # phase 3: out tiles rotate [P0,P1,P2,P6,P3] -- P4/P5 stay with the
        #          last attention group's AV pair so phase 3 never waits on
        #          its norm chain
        SUMP = P[7]
        BANKS3 = [0, 1, 2, 6, 3]

        def avp(g, dcc):
            return P[[6, 3][dcc] if g % 2 == 0 else [4, 5][dcc]]

        # ---- semaphores ----
        sems = {}
        for name in (
            "s_wq", "s_wq0a",
            "s_x00", "s_x01", "s_x02", "s_x03",
            "s_x04", "s_x05", "s_x06", "s_x07",
            "s_wv", "s_xt1", "s_xt2", "s_xt3",
            "s_cs0", "s_cs1", "s_cs2", "s_cs3",
            "s_init", "s_wo", "s_misc", "s_pq", "s_pv", "s_pqd", "s_vcp",
            "s_dve", "s_stp", "s_exp", "s_ptc", "s_sum", "s_av",
            "s_rc", "s_rcp", "s_norm", "s_p3",
            "s_b30", "s_b31", "s_b32", "s_b33", "s_b34",
            "s_out0", "s_out1", "s_out2", "s_out3",
        ):
            sems[name] = ctx.enter_context(nc.semaphore(name))
        S = types.SimpleNamespace(**sems)
        s_x0 = [S.s_x00, S.s_x01, S.s_x02, S.s_x03,
                S.s_x04, S.s_x05, S.s_x06, S.s_x07]
        s_xt = [None, S.s_xt1, S.s_xt2, S.s_xt3]
        s_cs = [S.s_cs0, S.s_cs1, S.s_cs2, S.s_cs3]
        s_out = [S.s_out0, S.s_out1, S.s_out2, S.s_out3]
        # phase-3 copy-done sems, one per rotation bank (engine-agnostic)
        s_b3 = [S.s_b30, S.s_b31, S.s_b32, S.s_b33, S.s_b34]
        # phase-3 copy engine assignment: ACT takes the first 10 (DVE is
        # still draining the last two softmax-tail chains then) + odd q;
        # DVE takes even q >= 10
        P3_ON_ACT = [q for q in range(96) if q < 10 or q % 2 == 1]
        P3_ON_DVE = [q for q in range(96) if q >= 10 and q % 2 == 0]

        with nc.Block() as block:

            # ---------------- SYNC: weights + xt b1-3 + stores ----------------
            @block.sync
            def _(sync):
                def ld(sem, out_ap, in_ap):
                    sync.dma_start(out_ap, in_ap).then_inc(sem, 16)

                def xt_batch(b, sem):
                    # one DMA: per-partition 24KB fully contiguous
                    ld(sem, xt_sb[:, b % 2, :, :], xt[:, b, :, :, :])

                def cs_batch(b):
                    tsl = slice(512 * b, 512 * b + 512)
                    ld(s_cs[b], cos_sb[:, tsl], cosT[:, tsl])
                    ld(s_cs[b], sin_sb[:, tsl], sinT[:, tsl])

                # prologue: the b0-critical stream (wqk slabs + b0 x pieces,
                # consumed pairwise by the piece-major qk loop) runs alone on
                # the sync queue at full bandwidth; wv/cos0/ident/wo go on
                # the ACT HWDGE queue, gated until most of b0 has landed.
                for s in range(8):
                    if s == 0:
                        # split slab 0: the PE starts on (m0, piece0) after
                        # ~0.5MB instead of ~1MB
                        ld(S.s_wq0a, wqk_sb[:, 0, 0:1, :, :],
                           wqk[:, 0, 0:1, :, :])
                        sync.dma_start(
                            xt_sb[:, 0, 0:3, :], xt[:, 0, 0, :, :],
                        ).then_inc(s_x0[0], 16)
                        ld(S.s_wq, wqk_sb[:, 0, 1:6, :, :],
                           wqk[:, 0, 1:6, :, :])
                        continue
                    ld(S.s_wq, wqk_sb[:, s, :, :, :], wqk[:, s, :, :, :])
                    sync.dma_start(
                        xt_sb[:, 0, 3 * s:3 * s + 3, :],
                        xt[:, 0, s, :, :],
                    ).then_inc(s_x0[s], 16)
                # don't let the 3MB b1 load flood the queue before b0 is in
                sync.wait_ge(S.s_wq, 16 * 8)
                xt_batch(1, s_xt[1])
                cs_batch(1)
                sync.wait_ge(S.s_pq, 6)
                sync.wait_ge(S.s_pv, 4)
                xt_batch(2, s_xt[2])
                cs_batch(2)
                sync.wait_ge(S.s_pq, 12)
                sync.wait_ge(S.s_pv, 8)
                xt_batch(3, s_xt[3])
                cs_batch(3)

                # phase 3 output stores
                for q in range(96):
                    sync.wait_ge(s_b3[q % 5], q // 5 + 1)
                    n, tc = q // 16, TCORD[q % 16]
                    sync.dma_start(
                        out[128 * tc:128 * tc + 128, 512 * n:512 * n + 512],
                        outst[:, q % 4, :],
                    ).then_inc(s_out[q % 4], 16)

            # ---------------- GPSIMD ----------------
            @block.gpsimd
            def _(gp):
                gp.memset(ones_sb[:, :], 1.0).then_inc(S.s_misc, 1)

            # ---------------- TENSOR (PE) ----------------
            @block.tensor
            def _(pe):
                # phase 2 tile helpers (defined first: the last 3 v groups
                # of phase 1 interleave the first S^T tiles)
                def emit_st(idx):
                    h, j, i = TILES[idx]
                    o = OFFS[idx]
                    diag = i >= 4 * j
                    if idx == 0:
                        # P0/P1 bank WAR vs RoPE pair 10 (reads qk banks
                        # P0/P1 of g20/g21); also covers all j0 q/k data
                        pe.wait_ge(S.s_pqd, 22)
                        pe.wait_ge(S.s_init, 16 * 2)  # ident + negtri
                    elif idx == 2:
                        # P2 bank WAR vs RoPE pair 11 (the last pair)
                        pe.wait_ge(S.s_pqd, 24)
                    if idx >= 3:
                        # ST bank WAR vs exp(idx-3); the wait value idx-2
                        # also covers cons(idx-3)'s exp data dependency
                        pe.wait_ge(S.s_exp, idx - 2)
                    for dc in range(2):
                        ins = pe.matmul(
                            P[idx % 3][:, o:512],
                            lhsT=qk_sb[:, 4 + dc, 128 * i:128 * i + 128],
                            rhs=qk_sb[:, 2 * h + dc, 512 * j + o:512 * j + 512],
                            start=(dc == 0),
                            stop=(dc == 1 and not diag),
                        )
                    if diag:
                        # add -30000 to the causal-masked triangle so exp
                        # underflows to zero -- replaces the DVE mask multiply
                        ins = pe.matmul(
                            P[idx % 3][:, o:o + 128],
                            lhsT=id_sb[:, :],
                            rhs=ntri_sb[:, :],
                            start=False,
                            stop=True,
                        )
                    ins.then_inc(S.s_stp, 1)

                # phase 1, b0: piece-major (s outer, all six m inner, banks
                # P0-P5) -- the PE starts on (slab 0, piece 0) after ~1MB
                # and the whole 7.9MB b0 load overlaps b0's 30.7us of qk
                # compute instead of serializing ahead of it
                for s in range(8):
                    pe.wait_ge(s_x0[s], 16)
                    if s > 0:
                        pe.wait_ge(S.s_wq, 16 * (s + 1))
                    for m in range(6):
                        if s == 0:
                            if m == 0:
                                pe.wait_ge(S.s_wq0a, 16)
                            elif m == 1:
                                pe.wait_ge(S.s_wq, 16)
                        for k3 in range(3):
                            ins = pe.matmul(
                                P[m][:, :],
                                lhsT=wqk_sb[:, s, m, k3, :],
                                rhs=xt_sb[:, 0, 3 * s + k3, :],
                                start=(s == 0 and k3 == 0),
                                stop=(s == 7 and k3 == 2),
                            )
                        if s == 7:
                            ins.then_inc(S.s_pq, 1)
                # v(b0) on P6/P7 (free in phase 1; P0-P5 hold b0's qk until
                # the RoPE pairs drain them)
                for ts in range(4):
                    if ts == 0:
                        pe.wait_ge(S.s_wv, 16)
                    else:
                        pe.wait_ge(S.s_vcp, max(ts - 1, 0))
                    for kc in range(KC):
                        ins = pe.matmul(
                            P[6 + ts % 2][:, 0:256],
                            lhsT=xt_sb[:, 0, kc, 128 * ts:128 * ts + 128],
                            rhs=wv_sb[:, kc, :],
                            start=(kc == 0),
                            stop=(kc == KC - 1),
                        )
                    ins.then_inc(S.s_pv, 1)

                # phase 1, b1-3: group-major (weights fully resident)
                for b in range(1, 4):
                    for m in range(6):
                        g = 6 * b + m
                        # bank g%4 was read by the RoPE pair containing
                        # group g-4; that pair completes at s_pqd = g-2
                        # for even g (pair g-4,g-3) and g-3 for odd g
                        # (pair g-5,g-4)
                        pe.wait_ge(S.s_pqd, g - 2 if g % 2 == 0 else g - 3)
                        if m == 0:
                            pe.wait_ge(s_xt[b], 16)
                        for kc in range(KC):
                            ins = pe.matmul(
                                P[g % 4][:, :],
                                lhsT=wqk_sb[:, kc // 3, m, kc % 3, :],
                                rhs=xt_sb[:, b % 2, kc, :],
                                start=(kc == 0),
                                stop=(kc == KC - 1),
                            )
                        ins.then_inc(S.s_pq, 1)
                    for ts in range(4):
                        vg = 4 * b + ts
                        if b == 1 and ts == 0:
                            # P4/P5 were b0's k-groups: RoPE pair 2 must
                            # have drained them before v(b1) overwrites
                            pe.wait_ge(S.s_pqd, 6)
                        pe.wait_ge(S.s_vcp, vg - 1)
                        for kc in range(KC):
                            ins = pe.matmul(
                                P[4 + vg % 2][:, 0:256],
                                lhsT=xt_sb[:, b % 2, kc, 128 * ts:128 * ts + 128],
                                rhs=wv_sb[:, kc, :],
                                start=(kc == 0),
                                stop=(kc == KC - 1),
                            )
                        ins.then_inc(S.s_pv, 1)
                        # hoist the first 3 attention S^T tiles between b3's
                        # v groups: their exps fill the ACT pipeline while
                        # the PE finishes phase 1, so the cons loop starts
                        # with zero exp-latency bubble
                        if b == 3:
                            if ts == 0:
                                emit_st(0)
                                emit_st(1)
                            elif ts == 1:
                                emit_st(2)

                def emit_sum(idx):
                    # SUM of tile idx is deferred one tile so the previous
                    # group's SUM-bank evacuation never blocks the PE; it is
                    # also the last reader of pt slot idx (-> s_ptc)
                    g = GROUP_OF[idx]
                    o = OFFS[idx]
                    first = idx == G_FIRST[g]
                    last = idx == G_LAST[g]
                    if idx == 0:
                        pe.wait_ge(S.s_misc, 1)  # ones_sb memset
                    if first:
                        pe.wait_ge(S.s_rc, g)  # SUM bank free (g=0 trivial)
                    sm = pe.matmul(
                        SUMP[:, o:512], lhsT=ones_sb[:, :],
                        rhs=pt_sb[:, idx % 4, o:512],
                        start=first, stop=last,
                    )
                    # one sem update per instruction: group-last SUM signals
                    # s_sum (softmax tail); others signal s_ptc (pt slot)
                    if last:
                        sm.then_inc(S.s_sum, 1)
                    else:
                        sm.then_inc(S.s_ptc, 1)

                vcp_seen = [0]

                def emit_cons(idx):
                    h, j, i = TILES[idx]
                    o = OFFS[idx]
                    g = GROUP_OF[idx]
                    first = idx == G_FIRST[g]
                    last = idx == G_LAST[g]
                    if idx + 3 >= N_TILES:
                        # no emit_st carries this tile's exp wait
                        pe.wait_ge(S.s_exp, idx + 1)
                    if idx == 0:
                        need = 4  # v tiles 0-3 (g0 is all of j0)
                    elif idx == 4:
                        need = 16  # P4/P5 bank WAR vs the b3 v-bank copies
                    else:
                        need = i + 1  # v_sb tile i data
                    if need > vcp_seen[0]:
                        pe.wait_ge(S.s_vcp, need)
                        vcp_seen[0] = need
                    if first and g >= 2:
                        pe.wait_ge(S.s_norm, 2 * g - 2)  # AV pair free
                    pt = pt_sb[:, idx % 4, o:512]
                    av = [
                        pe.matmul(
                            avp(g, dc)[:, o:512],
                            lhsT=v_sb[:, i, 128 * dc:128 * dc + 128],
                            rhs=pt,
                            start=first,
                            stop=last,
                        )
                        for dc in range(2)
                    ]
                    if last:
                        av[1].then_inc(S.s_av, 1)

                # st(0..2) were interleaved into phase 1's b3 v groups
                for idx in range(N_TILES):
                    if idx + 3 < N_TILES:
                        emit_st(idx + 3)
                    emit_cons(idx)
                    if idx >= 1:
                        emit_sum(idx - 1)
                    if idx == N_TILES - 1:
                        emit_sum(idx)

                # phase 3: local o_proj (K=512) over all 2048 tokens,
                # j3-first token order, 5-bank rotation [P0,P1,P2,P6,P3]
                for q in range(96):
                    n, tc = q // 16, TCORD[q % 16]
                    # data: tile tc needs groups GIDX[(0,j)], GIDX[(1,j)]
                    # for j = tc//4: j3 -> norm 10, j2 -> 12, j1 -> 14,
                    # j0 -> 16.  banks: P6/P3 are g6's AV pair (free at
                    # s_norm 13/14); P4/P5 (g7's) are never used here.
                    if q == 0:
                        pe.wait_ge(S.s_norm, 10)
                        pe.wait_ge(S.s_wo, 16)
                    elif q == 3:
                        pe.wait_ge(S.s_norm, 13)
                    elif q == 4:
                        pe.wait_ge(S.s_norm, 14)
                    elif q == 12:
                        pe.wait_ge(S.s_norm, 16)
                    if q >= 5:
                        # bank reuse: tile q-5's copy must have evacuated it
                        pe.wait_ge(s_b3[q % 5], q // 5)
                    bank = P[BANKS3[q % 5]]
                    for c2 in range(4):
                        h, dcc = divmod(c2, 2)
                        ins = pe.matmul(
                            bank[:, :],
                            lhsT=ot_sb[:, GIDX[(h, tc // 4)], dcc,
                                       128 * (tc % 4):128 * (tc % 4) + 128],
                            rhs=wo_sb[:, c2, 512 * n:512 * n + 512],
                            start=(c2 == 0),
                            stop=(c2 == 3),
                        )
                    ins.then_inc(S.s_p3, 1)

            # ---------------- VECTOR (DVE) ----------------
            @block.vector
            def _(ve):
                dvec = [0]  # same-engine serialization counter for temps

                def step(fn, *args, inc=None, inc_by=1):
                    if dvec[0]:
                        ve.wait_ge(S.s_dve, dvec[0])
                    ins = fn(*args)
                    if inc is None:
                        ins.then_inc(S.s_dve, 1)
                        dvec[0] += 1
                    else:
                        ins.then_inc(inc, inc_by)

                # phase 1: RoPE + v copies
                def rope_pair(b, p):
                    tsl = slice(512 * b, 512 * b + 512)
                    m = 2 * p
                    g0, g1 = 6 * b + m, 6 * b + m + 1
                    ve.wait_ge(S.s_pq, g1 + 1)
                    if p == 0:
                        ve.wait_ge(s_cs[b], 16 * 2)
                    if not (b == 0 and p == 0):
                        # tmpA/tmpB WAR vs the previous pair's final add
                        # (which increments s_pqd): DVE ops can pipeline,
                        # so an explicit wait is required
                        ve.wait_ge(S.s_pqd, 2 * (3 * b + p))
                    if b == 0:
                        # piece-major b0: group m lives in bank P[m]
                        q1, q2 = P[m][:, :], P[m + 1][:, :]
                    else:
                        q1, q2 = P[g0 % 4][:, :], P[g1 % 4][:, :]
                    step(ve.tensor_mul, tmpA[:, :], q1, cos_sb[:, tsl])
                    step(ve.tensor_mul, tmpB[:, :], q2, sin_sb[:, tsl])
                    step(ve.tensor_sub, qk_sb[:, m, tsl], tmpA[:, :],
                         tmpB[:, :])
                    step(ve.tensor_mul, tmpA[:, :], q2, cos_sb[:, tsl])
                    step(ve.tensor_mul, tmpB[:, :], q1, sin_sb[:, tsl])
                    step(ve.tensor_add, qk_sb[:, m + 1, tsl], tmpA[:, :],
                         tmpB[:, :], inc=S.s_pqd, inc_by=2)

                def v_copy(vg):
                    bank = P[6 + vg % 2] if vg < 4 else P[4 + vg % 2]
                    ve.wait_ge(S.s_pv, vg + 1)
                    ve.tensor_copy(v_sb[:, vg, :], bank[:, 0:256]).then_inc(
                        S.s_vcp, 1
                    )

                # b0: its v phase follows qk immediately (all RoPE becomes
                # runnable at once), so interleave copies with the pairs or
                # the P6/P7 double-buffer stalls the PE.  b1/b2: copies sit
                # between pair1 and pair2 for the same reason.  b3: pairs
                # first -- the hoisted st(2) needs pair 11 as early as
                # possible, and the interleaved S^T tiles give the DVE slack.
                rope_pair(0, 0)
                v_copy(0)
                v_copy(1)
                rope_pair(0, 1)
                v_copy(2)
                v_copy(3)
                rope_pair(0, 2)
                for b in (1, 2):
                    rope_pair(b, 0)
                    rope_pair(b, 1)
                    v_copy(4 * b + 0)
                    v_copy(4 * b + 1)
                    rope_pair(b, 2)
                    v_copy(4 * b + 2)
                    v_copy(4 * b + 3)
                rope_pair(3, 0)
                rope_pair(3, 1)
                rope_pair(3, 2)
                for ts in range(4):
                    v_copy(12 + ts)

                # phase 2: per-group softmax tail.  The copy evacuates the
                # SUM bank fast (s_rc) so the next group's SUMs never wait;
                # the slow reciprocal (3.4us) runs off the PE critical path.
                # The descending-j h1 group order guarantees each group's
                # chain fits inside the next group's PE time, so s_norm is
                # always ready when the AV pair is reused two groups later.
                for g in range(8):
                    ve.wait_ge(S.s_sum, g + 1)
                    ve.tensor_copy(cs_sb[:, g % 2, :], SUMP[:, :]).then_inc(
                        S.s_rc, 1
                    )
                    ve.wait_ge(S.s_rc, g + 1)  # cs RAW vs the copy above
                    ve.reciprocal(rb_sb[:, g % 2, :],
                                  cs_sb[:, g % 2, :]).then_inc(S.s_rcp, 1)
                    ve.wait_ge(S.s_av, g + 1)
                    ve.wait_ge(S.s_rcp, g + 1)  # rb RAW vs the reciprocal
                    ve.tensor_mul(ot_sb[:, g, 0, :], avp(g, 0)[:, :],
                                  rb_sb[:, g % 2, :]).then_inc(S.s_norm, 1)
                    ve.tensor_mul(ot_sb[:, g, 1, :], avp(g, 1)[:, :],
                                  rb_sb[:, g % 2, :]).then_inc(S.s_norm, 1)

                # phase 3: DVE's share of the output copies
                for q in P3_ON_DVE:
                    ve.wait_ge(S.s_p3, q + 1)
                    if q >= 4:
                        ve.wait_ge(s_out[q % 4], 16 * (q // 4))
                    ve.tensor_copy(outst[:, q % 4, :],
                                   P[BANKS3[q % 5]][:, :]).then_inc(
                        s_b3[q % 5], 1
                    )

            # ---------------- SCALAR (ACT): bulk DMAs + exp + copies -------
            @block.scalar
            def _(sc):
                # prologue: wv (needed at v(b0), right after the qk slabs)
                # and cos/sin b0 (needed at RoPE pair 0) on the ACT HWDGE
                # queue, gated so they only take bandwidth from the last
                # two slab/piece pairs
                sc.wait_ge(s_x0[5], 16)
                sc.dma_start(cos_sb[:, 0:512], cosT[:, 0:512]).then_inc(
                    s_cs[0], 16)
                sc.dma_start(sin_sb[:, 0:512], sinT[:, 0:512]).then_inc(
                    s_cs[0], 16)
                sc.dma_start(wv_sb[:, :, :], wv[:, :, :]).then_inc(S.s_wv, 16)
                sc.dma_start(id_sb[:, :], ident[:, :]).then_inc(S.s_init, 16)
                sc.dma_start(ntri_sb[:, :], negtri[:, :]).then_inc(S.s_init, 16)
                sc.dma_start(
                    wo_sb[:, :, :],
                    wo[:, :].rearrange("(c p) n -> p c n", p=128),
                ).then_inc(S.s_wo, 16)

                # cumulative count of non-group-last tiles (s_ptc increments)
                ptc_at = []
                c = 0
                for t in range(N_TILES):
                    if t != G_LAST[GROUP_OF[t]]:
                        c += 1
                    ptc_at.append(c)
                for idx in range(N_TILES):
                    o = OFFS[idx]
                    sc.wait_ge(S.s_stp, idx + 1)
                    if idx >= 4:
                        lo = idx - 4  # pt slot owner
                        if lo == G_LAST[GROUP_OF[lo]]:
                            sc.wait_ge(S.s_sum, GROUP_OF[lo] + 1)
                        else:
                            sc.wait_ge(S.s_ptc, ptc_at[lo])
                    sc.activation(
                        pt_sb[:, idx % 4, o:512],
                        P[idx % 3][:, o:512],
                        mybir.ActivationFunctionType.Exp,
                        scale=0.0625,
                    ).then_inc(S.s_exp, 1)

                # phase 3: ACT's share of the output copies
                for q in P3_ON_ACT:
                    sc.wait_ge(S.s_p3, q + 1)
                    if q >= 4:
                        sc.wait_ge(s_out[q % 4], 16 * (q // 4))
                    sc.copy(outst[:, q % 4, :],
                            P[BANKS3[q % 5]][:, :]).then_inc(
                        s_b3[q % 5], 1
                    )

    return nc


# ---------------- host side ----------------

NUM_HEADS = 16
NUM_KV_HEADS = 8
HEAD_DIM = 256
ROPE_THETA = 10000.0


def _prep(x, W_qkv, W_o):
    bf = ml_dtypes.bfloat16
    # xt[p, b, s, c, t] = x[512b+t, 384s+128c+p]: partition-major so each
    # (b, s) piece is a single 3KB-contiguous run per partition
    xt = np.ascontiguousarray(
        x.reshape(4, 512, 8, 3, 128).transpose(4, 0, 2, 3, 1)
    ).astype(bf)

    pos = np.arange(T, dtype=np.float64)
    inv_freq = 1.0 / ROPE_THETA ** (
        np.arange(0, HEAD_DIM, 2, dtype=np.float64) / HEAD_DIM
    )
    freqs = pos[:, None] * inv_freq[None, :]  # [T, 128]
    cosT = np.ascontiguousarray(np.cos(freqs).T).astype(np.float32)
    sinT = np.ascontiguousarray(np.sin(freqs).T).astype(np.float32)

    p = np.arange(128)[:, None]
    f = np.arange(128)[None, :]
    ident = np.eye(128, dtype=np.float32).astype(bf)
    negtri = np.where(f < p, -30000.0, 0.0).astype(np.float32).astype(bf)

    in_maps = []
    for c in range(N_CORES):
        q_cols = np.r_[
            HEAD_DIM * c:HEAD_DIM * (c + 1),
            HEAD_DIM * (c + 8):HEAD_DIM * (c + 9),
        ]
        k_cols = np.arange(
            HEAD_DIM * NUM_HEADS + HEAD_DIM * c,
            HEAD_DIM * NUM_HEADS + HEAD_DIM * (c + 1),
        )
        v_cols = np.arange(
            HEAD_DIM * (NUM_HEADS + NUM_KV_HEADS) + HEAD_DIM * c,
            HEAD_DIM * (NUM_HEADS + NUM_KV_HEADS) + HEAD_DIM * (c + 1),
        )
        # partition-major shuffles for long contiguous DMA lines:
        # wqk[p, s, m, k3, col] = W[128*(3s+k3)+p, 128m+col] (slab-major);
        # wv[p, c, col] = Wv[128c+p, col]
        wqk = np.ascontiguousarray(
            W_qkv[:, np.r_[q_cols, k_cols]]
            .reshape(KC, 128, 6, 128)
            .transpose(1, 2, 0, 3)
            .reshape(128, 6, 8, 3, 128)
            .transpose(0, 2, 1, 3, 4)
        ).astype(bf)
        wvc = np.ascontiguousarray(
            W_qkv[:, v_cols].reshape(KC, 128, 256).transpose(1, 0, 2)
        ).astype(bf)
        woc = np.ascontiguousarray(
            W_o[np.r_[HEAD_DIM * c:HEAD_DIM * (c + 1),
                      HEAD_DIM * (c + 8):HEAD_DIM * (c + 9)], :]
        ).astype(bf)
        in_maps.append(
            {
                "wqk": wqk,
                "wv": wvc,
                "wo": woc,
                "xt": xt,
                "cosT": cosT,
                "sinT": sinT,
                "ident": ident,
                "negtri": negtri,
            }
        )
    return in_maps


_CACHE = {}


def kernel(x, W_qkv, W_o):
    trace = bool(int(os.environ.get("KERNEL_TRACE", "0")))
    in_maps = _prep(
        np.asarray(x, np.float32),
        np.asarray(W_qkv, np.float32),
        np.asarray(W_o, np.float32),
    )
    if "nc" not in _CACHE:
        _CACHE["nc"] = build_program()
    nc = _CACHE["nc"]
    res = run_bass_kernel_spmd(
        nc, in_maps, list(range(N_CORES)), trace=trace,
        trace_cores=[0] if trace else None,
    )
    if trace:
        print(f"HW exec time: {res.exec_time_ns} ns")
        _CACHE["last_result"] = res
    acc = np.zeros((T, HID), dtype=np.float32)
    for c in range(N_CORES):
        acc += np.asarray(res.results[c]["out"], dtype=np.float32)
    return acc


if __name__ == "__main__":
    rng = np.random.default_rng(0)
    x = rng.standard_normal((T, HID), dtype=np.float32)
    Wq = (rng.standard_normal((HID, 8192), dtype=np.float32) * HID ** -0.5)
    Wo = (rng.standard_normal((4096, HID), dtype=np.float32) * 4096 ** -0.5)
    y = kernel(x, Wq, Wo)
    print("ran:", y.shape, y.dtype)

